# revision 2
# baseline (speedup 1.0000x reference)
"""Trainium2 Bass kernel v2 for nn_AttModel (B=8, S=96, D=768, R=24, RSEQ=8, TAG=3).

Data-parallel over batch: core i handles sample i.

Per-core structure:
  - weights host-cast bf16 + packed (proj_W 2 DMAs, rel_W 1 DMA)
  - refine scan (24 seq softmax steps, f32, in PSUM score space) with the
    normalize folded into tiny [8,8] stationary rescales (G'=G*rinv rows,
    D'=diag(rinv)); wsum accumulates on the PE.
  - H = H0(b0) + AW.T-slice @ wsum (rank-8 correction); H0 feature-major
    chains + AW computed on the PE in the scan's shadow (emitted for scan
    steps >= 8, after the W halves have landed).
  - main loop: V tiles [128,384] bf16 = relu(ht + hh_i); DVE_MS tiles on
    DVE (tensor_scalar add+max), ACT_MS tiles on ACT reading PSUM-resident
    ht (ScalarE activation Relu, bias=hh col); 18 MMs accumulate
    out[72,384]; output DMA'd straight from PSUM.

PSUM budget (8 banks, 1 bank per pool buffer):
  pso 2 | psr 1 (5 ht slices packed in one bank) | scanb 1 (s|wsum|G) |
  pstmp 4 (transposes/AW chunks/H0 chains/Delta ring)
"""
import sys

sys.path.insert(0, "/opt/trn_rl_repo")

import numpy as np

S, D, H3 = 96, 768, 2304
R, RSEQ, TAG, C = 24, 8, 3, 72
B = 8
KT = D // 128           # 6 k-tiles over D
MT = H3 // 128          # 18 m-tiles over 3D
IGRP = 4
NG = S // IGRP          # 24 groups
NFREE = IGRP * S        # 384
SCALE = 1.0 / float(np.sqrt(np.float32(D)))

N_ACT = 5               # m-tiles produced on ACT (PSUM-resident ht)
ACT_MS = list(range(MT - N_ACT, MT))
DVE_MS = list(range(MT - N_ACT))
AWC = 384               # AW psum chunk width
SHADOW_START = 8        # first scan step that gets shadow PE work
OUT_PSUM_DMA = False    # bass dma_start rejects PSUM source


def build_nc(repeat: int = 1):
    import concourse.bass as bass
    from concourse import bacc, mybir
    import concourse.tile as tile
    from concourse.masks import make_identity

    f32 = mybir.dt.float32
    bf16 = mybir.dt.bfloat16
    AF = mybir.ActivationFunctionType
    ALU = mybir.AluOpType
    AX = mybir.AxisListType

    nc = bacc.Bacc()
    enc = nc.dram_tensor("enc", [S, D], f32, kind="ExternalInput")
    arel = nc.dram_tensor("arel", [RSEQ, D], f32, kind="ExternalInput")
    pwbf = nc.dram_tensor("pwbf", [128, 2 * KT * H3], bf16, kind="ExternalInput")
    projb = nc.dram_tensor("projb", [H3], f32, kind="ExternalInput")
    rwbf = nc.dram_tensor("rwbf", [128, MT * C], bf16, kind="ExternalInput")
    out = nc.dram_tensor("out", [C, S * S], f32, kind="ExternalOutput")

    with tile.TileContext(nc) as tc:
        with (
            tc.tile_pool(name="persist", bufs=1) as pp,
            tc.tile_pool(name="work", bufs=4) as wp,
            tc.tile_pool(name="vd", bufs=20) as vdp,
            tc.tile_pool(name="va", bufs=10) as vap,
            tc.tile_pool(name="pso", bufs=2, space="PSUM") as pso,
            tc.tile_pool(name="psr", bufs=1, space="PSUM") as psrp,
            tc.tile_pool(name="psss", bufs=1, space="PSUM") as psss,
            tc.tile_pool(name="pssw", bufs=1, space="PSUM") as pssw,
            tc.tile_pool(name="pstmp", bufs=3, space="PSUM") as pstmp,
        ):

            def body(_it=None):
                # ---------- loads ----------
                ident = pp.tile([128, 128], f32, tag="ident")
                make_identity(nc, ident[:])

                enc_nat = pp.tile([S, D], f32, tag="enc_nat")
                nc.sync.dma_start(enc_nat[:], enc[:])
                a_nat = pp.tile([RSEQ, D], f32, tag="a_nat")
                nc.sync.dma_start(a_nat[:], arel[:])
                pb_sb = pp.tile([128, MT], f32, tag="pb")
                nc.sync.dma_start(
                    pb_sb[:], projb.rearrange("(t p) -> p t", p=128)
                )
                rw_all = pp.tile([128, MT * C], bf16, tag="rw_all")
                nc.sync.dma_start(rw_all[:], rwbf[:])
                pw_h = pp.tile([128, KT * H3], bf16, tag="pw_h")
                nc.sync.dma_start(pw_h[:], pwbf[:, : KT * H3])
                pw_t = pp.tile([128, KT * H3], bf16, tag="pw_t")
                nc.sync.dma_start(pw_t[:], pwbf[:, KT * H3:])

                def pw(kt):
                    src = pw_h if kt < KT else pw_t
                    k = kt % KT
                    return src[:, k * H3:(k + 1) * H3]

                def rwr(m):
                    return rw_all[:, m * C:(m + 1) * C]

                # PSUM tiles: every open matmul chain owns a full bank
                # (start=True zeroes the whole 2KB zero-region).
                s_full = psss.tile([RSEQ, S], f32, tag="s", bufs=1)
                s_ps = s_full[:]
                w_full = pssw.tile([RSEQ, S], f32, tag="w", bufs=1)
                wsum_ps = w_full[:]
                # psr bank holds finalized ht data for ACT_MS; it is never a
                # matmul target (written by DVE tensor ops only).
                psr_all = psrp.tile([128, N_ACT * S], f32, tag="psr", bufs=1)
                psr_ht = {
                    m: psr_all[:, i * S:(i + 1) * S]
                    for i, m in enumerate(ACT_MS)
                }

                # ---------- transposes ----------
                bT_f, bT_b = [], []
                for k in range(KT):
                    ps = pstmp.tile([128, 384], f32, tag="tmp")
                    nc.tensor.transpose(
                        ps[:, :S],
                        enc_nat[:, k * 128:(k + 1) * 128], ident[:S, :S]
                    )
                    tf = pp.tile([128, S], f32, tag=f"bTf{k}")
                    nc.scalar.copy(tf[:], ps[:, :S])
                    tb = pp.tile([128, S], bf16, tag=f"bTb{k}")
                    nc.vector.tensor_scalar_mul(tb[:], ps[:, :S], 1.0)
                    bT_f.append(tf)
                    bT_b.append(tb)
                at_scl, at_bf = [], []
                at_raw = []
                for k in range(KT):
                    ps = pstmp.tile([128, 384], f32, tag="tmp")
                    nc.tensor.transpose(
                        ps[:, :RSEQ], a_nat[:, k * 128:(k + 1) * 128],
                        ident[:RSEQ, :RSEQ],
                    )
                    tr = pp.tile([128, RSEQ], f32, tag=f"atr{k}")
                    nc.scalar.copy(tr[:], ps[:, :RSEQ])
                    ts = pp.tile([128, RSEQ], f32, tag=f"ats{k}")
                    nc.scalar.mul(ts[:], ps[:, :RSEQ], SCALE)
                    tbf = pp.tile([128, RSEQ], bf16, tag=f"atb{k}")
                    nc.vector.tensor_scalar_mul(tbf[:], ps[:, :RSEQ], 1.0)
                    at_raw.append(tr)
                    at_scl.append(ts)
                    at_bf.append(tbf)

                # G = scale * A @ A.T (symmetric) [8, 8] via tmp ring
                gtmp = pstmp.tile([128, 384], f32, tag="tmp")
                for k in range(KT):
                    nc.tensor.matmul(
                        gtmp[:RSEQ, :RSEQ], at_scl[k][:], at_raw[k][:],
                        start=(k == 0), stop=(k == KT - 1),
                        skip_group_check=True,
                    )
                g_sb = pp.tile([RSEQ, RSEQ], f32, tag="g")
                nc.vector.tensor_scalar_mul(g_sb[:], gtmp[:RSEQ, :RSEQ], 1.0)
                i8 = pp.tile([RSEQ, RSEQ], f32, tag="i8")
                make_identity(nc, i8[:])

                # s0 = scale * A @ b0.T
                for k in range(KT):
                    nc.tensor.matmul(
                        s_ps, at_scl[k][:], bT_f[k][:],
                        start=(k == 0), stop=False, skip_group_check=True,
                    )

                # ---------- shadow PE work (AW + H0), emitted inside scan --
                aw_sb = pp.tile([RSEQ, 2 * H3], bf16, tag="aw")
                hh0 = [None] * MT
                ht0 = [None] * MT
                shadow = []

                def emit_aw(half, c0, cw):
                    def go():
                        ps = pstmp.tile([128, 384], f32, tag="tmp")
                        for k in range(KT):
                            nc.tensor.matmul(
                                ps[:RSEQ, :cw],
                                at_bf[k][:],
                                (pw_h if half == 0 else pw_t)[
                                    :, k * H3 + c0: k * H3 + c0 + cw
                                ],
                                start=(k == 0), stop=(k == KT - 1),
                                skip_group_check=True,
                            )
                        nc.vector.tensor_scalar_mul(
                            aw_sb[:, half * H3 + c0: half * H3 + c0 + cw],
                            ps[:RSEQ, :cw], 1.0,
                        )
                    return go

                def emit_h0h(m):
                    def go():
                        ps = pstmp.tile([128, 384], f32, tag="tmp")
                        msl = slice(m * 128, (m + 1) * 128)
                        for k in range(KT):
                            nc.tensor.matmul(
                                ps[:, :S], pw(k)[:, msl], bT_b[k][:],
                                start=(k == 0), stop=(k == KT - 1),
                                skip_group_check=True,
                            )
                        t = pp.tile([128, S], f32, tag=f"hh0_{m}")
                        if m % 2 == 0:
                            nc.scalar.activation(
                                t[:], ps[:, :S], AF.Identity,
                                bias=pb_sb[:, m:m + 1], scale=1.0,
                            )
                        else:
                            nc.vector.tensor_scalar(
                                t[:], ps[:, :S], pb_sb[:, m:m + 1], None,
                                op0=ALU.add,
                            )
                        hh0[m] = t
                    return go

                def emit_h0t(m):
                    def go():
                        msl = slice(m * 128, (m + 1) * 128)
                        ps = pstmp.tile([128, 384], f32, tag="tmp")
                        for k in range(KT):
                            nc.tensor.matmul(
                                ps[:, :S], pw(KT + k)[:, msl], bT_b[k][:],
                                start=(k == 0), stop=(k == KT - 1),
                                skip_group_check=True,
                            )
                        t = pp.tile([128, S], f32, tag=f"ht0_{m}")
                        if m % 2 == 0:
                            nc.scalar.copy(t[:], ps[:, :S])
                        else:
                            nc.vector.tensor_scalar_mul(
                                t[:], ps[:, :S], 1.0
                            )
                        ht0[m] = t
                    return go

                for half in range(2):
                    for c0 in range(0, H3, AWC):
                        shadow.append(emit_aw(half, c0, min(AWC, H3 - c0)))
                for m in range(MT):
                    shadow.append(emit_h0h(m))
                for m in range(MT):
                    shadow.append(emit_h0t(m))

                shadow_iter = iter(shadow)

                def run_shadow(n):
                    for _ in range(n):
                        thunk = next(shadow_iter, None)
                        if thunk is None:
                            return
                        thunk()

                # ---------- refine scan ----------
                nsh = len(shadow)  # 48
                steps_with_shadow = R - SHADOW_START
                per_step = -(-nsh // steps_with_shadow)  # ceil
                for t in range(R):
                    negmax = wp.tile([RSEQ, 1], f32, tag="negmax")
                    nc.vector.reduce_max(
                        negmax[:], s_ps, axis=AX.X, negate=True
                    )
                    u = wp.tile([RSEQ, S], f32, tag="u")
                    rs = wp.tile([RSEQ, 1], f32, tag="rs")
                    nc.scalar.activation(
                        u[:], s_ps, AF.Exp, bias=negmax[:], scale=1.0,
                        accum_out=rs[:],
                    )
                    rinv = wp.tile([RSEQ, 1], f32, tag="rinv")
                    nc.vector.reciprocal(rinv[:], rs[:])
                    gp = wp.tile([RSEQ, RSEQ], f32, tag="gp")
                    nc.vector.tensor_scalar_mul(gp[:], g_sb[:], rinv[:])
                    dp = wp.tile([RSEQ, RSEQ], f32, tag="dp")
                    nc.vector.tensor_scalar_mul(dp[:], i8[:], rinv[:])
                    nc.tensor.matmul(
                        wsum_ps, dp[:], u[:],
                        start=(t == 0), stop=(t == R - 1),
                        skip_group_check=True,
                    )
                    if t < R - 1:
                        nc.tensor.matmul(
                            s_ps, gp[:], u[:],
                            start=False, stop=(t == R - 2),
                            skip_group_check=True,
                        )
                    if t >= SHADOW_START:
                        run_shadow(per_step)
                run_shadow(nsh)

                wsum_bf = pp.tile([RSEQ, S], bf16, tag="wsum_bf")
                nc.vector.tensor_scalar_mul(wsum_bf[:], wsum_ps, 1.0)

                # ---------- Delta + finalize ----------
                hh = [None] * MT
                ht = [None] * MT
                for m in range(MT):
                    msl = slice(m * 128, (m + 1) * 128)
                    dps = pstmp.tile([128, 384], f32, tag="tmp")
                    nc.tensor.matmul(
                        dps[:, :S], aw_sb[:, msl], wsum_bf[:],
                        start=True, stop=True, skip_group_check=True,
                    )
                    th = pp.tile([128, S], f32, tag=f"hh{m}")
                    nc.vector.tensor_tensor(
                        th[:], hh0[m][:], dps[:, :S], op=ALU.add
                    )
                    hh[m] = th
                    tsl = slice(H3 + m * 128, H3 + (m + 1) * 128)
                    dpt = pstmp.tile([128, 384], f32, tag="tmp")
                    nc.tensor.matmul(
                        dpt[:, :S], aw_sb[:, tsl], wsum_bf[:],
                        start=True, stop=True, skip_group_check=True,
                    )
                    if m in ACT_MS:
                        nc.vector.tensor_tensor(
                            psr_ht[m], ht0[m][:], dpt[:, :S], op=ALU.add
                        )
                    else:
                        tt = pp.tile([128, S], bf16, tag=f"ht{m}")
                        nc.vector.tensor_tensor(
                            tt[:], ht0[m][:], dpt[:, :S], op=ALU.add
                        )
                        ht[m] = tt

                # ---------- pairwise main loop ----------
                # The staging copy of group g is emitted after group g+1's
                # V-production so the in-order DVE/ACT queues never stall
                # on the PE chain of the current group (pso ring = 2).
                pend = []

                def flush_out():
                    g0, ops0 = pend.pop(0)
                    ostg = wp.tile([C, NFREE], f32, tag="ostg")
                    if g0 % 2 == 0:
                        nc.scalar.copy(ostg[:], ops0[:])
                    else:
                        nc.vector.tensor_scalar_mul(ostg[:], ops0[:], 1.0)
                    nc.sync.dma_start(
                        out[:, g0 * NFREE:(g0 + 1) * NFREE], ostg[:]
                    )

                for ig in range(NG):
                    ops = pso.tile([C, NFREE], f32, tag="ops")
                    vtiles = {}
                    for m in ACT_MS:
                        v = vap.tile([128, NFREE], bf16, tag="va")
                        for ii in range(IGRP):
                            i = ig * IGRP + ii
                            nc.scalar.activation(
                                v[:, ii * S:(ii + 1) * S], psr_ht[m],
                                AF.Relu, bias=hh[m][:, i:i + 1], scale=1.0,
                            )
                        vtiles[m] = v
                    for m in DVE_MS:
                        v = vdp.tile([128, NFREE], bf16, tag="vd")
                        for ii in range(IGRP):
                            i = ig * IGRP + ii
                            nc.vector.tensor_scalar(
                                v[:, ii * S:(ii + 1) * S], ht[m][:],
                                hh[m][:, i:i + 1], 0.0,
                                op0=ALU.add, op1=ALU.max,
                            )
                        vtiles[m] = v
                    order = DVE_MS + ACT_MS
                    for j, m in enumerate(order):
                        nc.tensor.matmul(
                            ops[:], rwr(m), vtiles[m][:],
                            start=(j == 0), stop=(j == MT - 1),
                        )
                    pend.append((ig, ops))
                    if len(pend) > 1:
                        flush_out()
                while pend:
                    flush_out()

            if repeat == 1:
                body()
            else:
                with tc.For_i(0, repeat, 1) as it:
                    body(it)

    nc.finalize()
    return nc


_CACHED_NC = None


def _prep_in_maps(encoded_text, rel_types_encoded, proj_W, proj_b, rel_W):
    import ml_dtypes

    relw_perm = np.ascontiguousarray(
        np.asarray(rel_W, np.float32).reshape(H3, R, TAG)
        .transpose(0, 2, 1).reshape(H3, C)
    )
    rw_pack = np.zeros((128, MT * C), np.float32)
    for m in range(MT):
        rw_pack[:, m * C:(m + 1) * C] = relw_perm[m * 128:(m + 1) * 128, :]
    rw_pack = rw_pack.astype(ml_dtypes.bfloat16)

    pw = np.asarray(proj_W, np.float32)
    pw_pack = np.zeros((128, 2 * KT * H3), np.float32)
    for kt in range(2 * KT):
        pw_pack[:, kt * H3:(kt + 1) * H3] = pw[kt * 128:(kt + 1) * 128, :]
    pw_pack = pw_pack.astype(ml_dtypes.bfloat16)

    in_maps = []
    for i in range(B):
        in_maps.append({
            "enc": np.ascontiguousarray(encoded_text[i], dtype=np.float32),
            "arel": np.ascontiguousarray(
                rel_types_encoded[i], dtype=np.float32
            ),
            "pwbf": pw_pack,
            "projb": np.ascontiguousarray(proj_b, dtype=np.float32),
            "rwbf": rw_pack,
        })
    return in_maps


def _assemble(results, rel_b):
    outs = []
    for i in range(B):
        o = results[i]["out"].reshape(TAG, R, S, S)
        outs.append(o)
    full = np.stack(outs, axis=0).astype(np.float32)
    if np.any(rel_b):
        relb_perm = np.asarray(rel_b, dtype=np.float32).reshape(R, TAG).T
        full = full + relb_perm[None, :, :, None, None]
    return full


def kernel(encoded_text, rel_types_encoded, proj_W, proj_b, rel_W, rel_b):
    global _CACHED_NC
    from concourse.bass_utils import run_bass_kernel_spmd

    if _CACHED_NC is None:
        _CACHED_NC = build_nc(repeat=1)
    in_maps = _prep_in_maps(
        encoded_text, rel_types_encoded, proj_W, proj_b, rel_W
    )
    res = run_bass_kernel_spmd(_CACHED_NC, in_maps, list(range(B)))
    return _assemble(res.results, rel_b)
